# revision 13
# baseline (speedup 1.0000x reference)
"""MHA kernel for TRN2, 8-core SPMD (head-parallel).

Sharding: each core owns 2 of the 16 heads (column slices of W_q/W_k/W_v,
row slice of W_o). Each core computes its heads' Q/K/V for all tokens,
causal attention (lower-triangular blocks only; softmax without max
subtraction since scores ~ N(0,1)), its A shard, and a partial y
(contraction over its 128 head dims). Host sums y partials and concatenates
A shards.

Matmuls run in float32r (tf32-class, ~1.5e-4 rel err, 4x the fp32 rate).
"""
import sys

sys.path.insert(0, "/opt/trn_rl_repo")

import numpy as np

B, L, D, H = 2, 2048, 1024, 16
DK = D // H          # 64
NC = 8               # cores
HPC = H // NC        # 2 heads per core
TOK = B * L          # 4096
CHUNK = 512          # token chunk (phase 1) / q chunk (phase 2)
KT = 128             # k tile
NQC = L // CHUNK     # 4 q-chunks per batch
NKTB = L // KT       # 16 k-tiles per batch

_prog = None


def _build():
    import concourse.bacc as bacc
    import concourse.mybir as mybir
    import concourse.tile as tile
    from concourse.masks import make_identity

    f32 = mybir.dt.float32
    f32r = mybir.dt.float32r
    bf16 = mybir.dt.bfloat16
    ptdt = f32r          # dtype of exp(P) tiles / PV operands
    Exp = mybir.ActivationFunctionType.Exp
    Copy = mybir.ActivationFunctionType.Copy

    nc = bacc.Bacc(None, target_bir_lowering=False)
    x = nc.dram_tensor("x", [TOK, D], f32, kind="ExternalInput")
    wq = nc.dram_tensor("wq", [D, NC * HPC * DK // NC], f32, kind="ExternalInput")
    wk = nc.dram_tensor("wk", [D, 128], f32, kind="ExternalInput")
    wv = nc.dram_tensor("wv", [D, 128], f32, kind="ExternalInput")
    wo = nc.dram_tensor("wo", [128, D], f32, kind="ExternalInput")
    mtri = nc.dram_tensor("mtri", [128, 128], f32, kind="ExternalInput")
    mtril = nc.dram_tensor("mtril", [128, 128], f32, kind="ExternalInput")
    A = nc.dram_tensor("A", [B, HPC, L, L], f32, kind="ExternalOutput")
    y = nc.dram_tensor("y", [TOK, D], f32, kind="ExternalOutput")

    with tile.TileContext(nc) as tc:
        with tc.tile_pool(name="consts", bufs=1) as consts, \
             tc.tile_pool(name="persist", bufs=1) as persist:
            identf = consts.tile([128, 128], f32)
            make_identity(nc, identf[:, :])
            identr = consts.tile([128, 128], f32r)
            nc.vector.tensor_copy(identr[:, :], identf[:, :])
            identp = consts.tile([128, 128], ptdt)
            nc.vector.tensor_copy(identp[:, :], identf[:, :])
            mtri_sb = consts.tile([128, 128], f32)
            nc.sync.dma_start(out=mtri_sb[:, :], in_=mtri.ap())
            mtril_sb = consts.tile([128, 128], f32)
            nc.sync.dma_start(out=mtril_sb[:, :], in_=mtril.ap())
            ones_f = consts.tile([1, 128], f32)
            nc.vector.memset(ones_f[:, :], 1.0)
            ones_r = consts.tile([1, 128], f32r)
            nc.vector.tensor_copy(ones_r[:, :], ones_f[:, :])
            ones_col = consts.tile([128, 1], f32)
            nc.vector.memset(ones_col[:, :], 1.0)

            # weights -> f32r  (lhsT layout: [k-tile-row 128, k-tile, cols])
            wq_r = persist.tile([128, 8, 128], f32r)
            wk_r = persist.tile([128, 8, 128], f32r)
            wv_r = persist.tile([128, 8, 128], f32r)
            wo_r = persist.tile([128, D], f32r)
            qt_sb = persist.tile([128, B, L], f32r)   # Q^T  (2 heads stacked)
            kt_sb = persist.tile([128, B, L], f32r)   # K^T
            v_sb = persist.tile([128, B, NKTB, HPC, DK + 1], ptdt)  # [V|1]

            with tc.tile_pool(name="p1sb", bufs=2) as p1sb, \
                 tc.tile_pool(name="p1ps", bufs=1, space="PSUM") as p1ps:
                for wdram, wr in ((wq, wq_r), (wk, wk_r), (wv, wv_r)):
                    wst = p1sb.tile([128, 8, 128], f32, tag="wst")
                    nc.sync.dma_start(
                        out=wst[:, :, :],
                        in_=wdram.ap().rearrange("(kt p) m -> p kt m", p=128),
                    )
                    nc.vector.tensor_copy(wr[:, :, :], wst[:, :, :])
                wst = p1sb.tile([128, 8, 128], f32, tag="wst")
                nc.sync.dma_start(
                    out=wst[:, :, :],
                    in_=wo.ap().rearrange("p (kt m) -> p kt m", kt=8),
                )
                nc.vector.tensor_copy(
                    wo_r.rearrange("p (kt m) -> p kt m", kt=8), wst[:, :, :]
                )

                # ones column of [V|1]
                for b in range(B):
                    for kt in range(NKTB):
                        for hh in range(HPC):
                            nc.vector.tensor_copy(
                                v_sb[:, b, kt, hh, DK:DK + 1], ones_col[:, :]
                            )

                # ---- phase 1: x -> x^T -> Q^T/K^T/V ----
                for ci in range(TOK // CHUNK):
                    b = ci // NQC
                    cb = ci % NQC  # chunk within batch
                    xst = p1sb.tile([128, 4, D], f32, tag="xst")
                    nc.sync.dma_start(
                        out=xst[:, :, :],
                        in_=x.ap()[ci * CHUNK:(ci + 1) * CHUNK, :].rearrange(
                            "(n p) d -> p n d", p=128
                        ),
                    )
                    xtc = p1sb.tile([128, 8, CHUNK], f32r, tag="xtc")
                    for kt in range(8):
                        for t in range(4):
                            tp = p1ps.tile([128, 128], f32, tag="tp", bufs=4)
                            nc.tensor.transpose(
                                tp[:, :], xst[:, t, kt * 128:(kt + 1) * 128],
                                identf[:, :],
                            )
                            eng = nc.scalar if (kt + t) % 2 else nc.vector
                            if eng is nc.scalar:
                                nc.scalar.copy(
                                    out=xtc[:, kt, t * 128:(t + 1) * 128],
                                    in_=tp[:, :],
                                )
                            else:
                                nc.vector.tensor_copy(
                                    xtc[:, kt, t * 128:(t + 1) * 128], tp[:, :]
                                )
                    for wr, dst in ((wq_r, qt_sb), (wk_r, kt_sb)):
                        pp = p1ps.tile([128, CHUNK], f32, tag="proj", bufs=3)
                        for kt in range(8):
                            nc.tensor.matmul(
                                pp[:, :], wr[:, kt, :], xtc[:, kt, :],
                                start=(kt == 0), stop=(kt == 7),
                            )
                        nc.scalar.copy(
                            out=dst[:, b, cb * CHUNK:(cb + 1) * CHUNK], in_=pp[:, :]
                        )
                    # V: project then transpose to [tokens, dims]
                    pp = p1ps.tile([128, CHUNK], f32, tag="proj", bufs=3)
                    for kt in range(8):
                        nc.tensor.matmul(
                            pp[:, :], wv_r[:, kt, :], xtc[:, kt, :],
                            start=(kt == 0), stop=(kt == 7),
                        )
                    vtmp = p1sb.tile([128, CHUNK], f32, tag="vtmp")
                    nc.vector.tensor_copy(vtmp[:, :], pp[:, :])
                    for t in range(4):
                        vp = p1ps.tile([128, 128], f32, tag="vtp")
                        nc.tensor.transpose(
                            vp[:, :], vtmp[:, t * 128:(t + 1) * 128], identf[:, :]
                        )
                        ktile = cb * 4 + t
                        for hh in range(HPC):
                            nc.vector.tensor_copy(
                                v_sb[:, b, ktile, hh, 0:DK],
                                vp[:, hh * DK:(hh + 1) * DK],
                            )

            # ---- phase 2: attention + Wo ----
            with tc.tile_pool(name="p2sb", bufs=1) as p2sb, \
                 tc.tile_pool(name="ptp", bufs=16) as ptp, \
                 tc.tile_pool(name="aasm", bufs=6) as aasm, \
                 tc.tile_pool(name="p2x", bufs=2) as p2x, \
                 tc.tile_pool(name="rqp", bufs=4) as rqp, \
                 tc.tile_pool(name="ysb", bufs=3) as ysbp, \
                 tc.tile_pool(name="p2ps", bufs=1, space="PSUM") as p2ps:
                for b in range(B):
                    for qc in range(NQC):
                        q0 = qc * CHUNK
                        nkt = 4 * (qc + 1)
                        otn = p2x.tile([128, CHUNK], f32r, tag="otn")
                        # stage A: S + exp, heads interleaved so consecutive S
                        # matmuls hit alternating PE row groups (0-63 / 64-127)
                        # and LDWEIGHTS can pull ahead of in-flight matmuls.
                        pts_h = [[], []]
                        for kt in range(nkt):
                            k0 = kt * KT
                            off = max(0, k0 - q0)
                            for h in range(HPC):
                                r0, r1 = h * DK, (h + 1) * DK
                                s_ps = p2ps.tile([128, CHUNK], f32, tag="sps", bufs=2)
                                nc.tensor.matmul(
                                    s_ps[:, off:CHUNK],
                                    kt_sb[r0:r1, b, k0:k0 + KT],
                                    qt_sb[r0:r1, b, q0 + off:q0 + CHUNK],
                                    start=True, stop=True,
                                )
                                pt = ptp.tile([128, CHUNK], ptdt, tag="pt", bufs=32)
                                nc.scalar.activation(
                                    out=pt[:, off:CHUNK], in_=s_ps[:, off:CHUNK],
                                    func=Exp, scale=0.125,
                                )
                                if k0 >= q0:  # diagonal tile: causal tri-mask
                                    nc.vector.tensor_mul(
                                        pt[:, off:off + KT], pt[:, off:off + KT],
                                        mtri_sb[:, :],
                                    )
                                pts_h[h].append(pt)
                        # stage B: PV, heads interleaved (alternating PSUM banks)
                        o_ps_h = []
                        for h in range(HPC):
                            o_ps = p2ps.tile([DK + 1, CHUNK], f32, tag="oy",
                                             bufs=2, name=f"ops_{b}_{qc}_{h}")
                            o_ps_h.append(o_ps)
                        for kt in range(nkt):
                            off = max(0, kt * KT - q0)
                            for h in range(HPC):
                                nc.tensor.matmul(
                                    o_ps_h[h][:, off:CHUNK],
                                    v_sb[:, b, kt, h, :],
                                    pts_h[h][kt][:, off:CHUNK],
                                    start=(kt == 0), stop=(kt == nkt - 1),
                                )
                        rq_h = [[], []]
                        rs_h = []
                        for h in range(HPC):
                            r0, r1 = h * DK, (h + 1) * DK
                            o_ps = o_ps_h[h]
                            zrow = p2x.tile([1, CHUNK], f32, tag="zrow")
                            nc.vector.tensor_copy(zrow[:, :], o_ps[DK:DK + 1, :])
                            r_sb = p2x.tile([1, CHUNK], f32, tag="rsb")
                            nc.vector.reciprocal_approx_fast(
                                out=r_sb[:, :], in_=zrow[:, :]
                            )
                            rs_h.append(r_sb)
                            r_r = p2x.tile([1, CHUNK], f32r, tag="rr")
                            nc.vector.tensor_copy(r_r[:, :], r_sb[:, :])
                            bc_ps = p2ps.tile([DK, CHUNK], f32, tag="bc")
                            nc.tensor.matmul(
                                bc_ps[:, :], ones_r[:, 0:DK], r_r[:, :],
                                start=True, stop=True,
                            )
                            bc_sb = p2x.tile([DK, CHUNK], f32, tag="bcsb")
                            nc.vector.tensor_copy(bc_sb[:, :], bc_ps[:, :])
                            nc.vector.tensor_mul(
                                otn[r0:r1, :], o_ps[0:DK, :], bc_sb[:, :]
                            )
                            for t in range(4):
                                zc_ps = p2ps.tile([128, CHUNK], f32, tag="sps",
                                                  bufs=2)
                                nc.tensor.matmul(
                                    zc_ps[:, 0:1], r_sb[:, t * 128:(t + 1) * 128],
                                    ones_f[:, 0:1], start=True, stop=True,
                                )
                                rq = rqp.tile([128, 1], f32, tag="rq", bufs=8)
                                nc.vector.tensor_copy(rq[:, :], zc_ps[:, 0:1])
                                rq_h[h].append(rq)
                        # stage C: A shard via S-natural recompute; normalize
                        # folded into exp as a per-partition ln(1/Z) bias.
                        for t in range(4):
                            a_asm_h = []
                            for h in range(HPC):
                                a_asm = aasm.tile([128, L], f32, tag="aasm",
                                                  name=f"aasm_{b}_{qc}_{t}_{h}")
                                a_asm_h.append(a_asm)
                            ncols = q0 + t * 128 + 128
                            qa, qb2 = q0 + t * 128, q0 + t * 128 + 128
                            for c0 in range(0, ncols, CHUNK):
                                cw = min(CHUNK, ncols - c0)
                                for h in range(HPC):
                                    r0, r1 = h * DK, (h + 1) * DK
                                    sn_ps = p2ps.tile([128, CHUNK], f32,
                                                      tag="snat", bufs=3,
                                                      name=f"sn_{h}")
                                    nc.tensor.matmul(
                                        sn_ps[:, 0:cw],
                                        qt_sb[r0:r1, b, qa:qb2],
                                        kt_sb[r0:r1, b, c0:c0 + cw],
                                        start=True, stop=True,
                                    )
                                    nc.scalar.activation(
                                        out=a_asm_h[h][:, c0:c0 + cw],
                                        in_=sn_ps[:, 0:cw],
                                        func=Exp, scale=0.125,
                                    )
                                    neng = nc.gpsimd if (c0 // CHUNK) % 2 == 0 \
                                        else nc.vector
                                    neng.tensor_scalar_mul(
                                        a_asm_h[h][:, c0:c0 + cw],
                                        a_asm_h[h][:, c0:c0 + cw],
                                        rq_h[h][t][:, :],
                                    )
                            for h in range(HPC):
                                # causal mask on the diagonal 128-block
                                nc.vector.tensor_mul(
                                    a_asm_h[h][:, qa:qb2],
                                    a_asm_h[h][:, qa:qb2], mtril_sb[:, :],
                                )
                                nc.sync.dma_start(
                                    out=A.ap()[b, h, qa:qb2, 0:ncols],
                                    in_=a_asm_h[h][:, 0:ncols],
                                )
                        # Wo: y tile = OtN.T @ wo_rows (partial over this core's dims)
                        for t in range(4):
                            for n in range(2):
                                y_ps = p2ps.tile([128, CHUNK], f32, tag="oy", bufs=2)
                                nc.tensor.matmul(
                                    y_ps[:, :],
                                    otn[:, t * 128:(t + 1) * 128],
                                    wo_r[:, n * CHUNK:(n + 1) * CHUNK],
                                    start=True, stop=True,
                                )
                                y_sb = ysbp.tile([128, CHUNK], f32, tag="ysb")
                                nc.vector.tensor_copy(y_sb[:, :], y_ps[:, :])
                                g0 = b * L + q0 + t * 128
                                nc.sync.dma_start(
                                    out=y.ap()[g0:g0 + 128,
                                               n * CHUNK:(n + 1) * CHUNK],
                                    in_=y_sb[:, :],
                                )
    nc.compile()
    return nc


def _get_prog():
    global _prog
    if _prog is None:
        _prog = _build()
    return _prog


def _reference_np(query, mask, W_q, W_k, W_v, W_o):
    b, l, d = query.shape
    q = (query @ W_q).reshape(b, l, H, DK).transpose(0, 2, 1, 3)
    k = (query @ W_k).reshape(b, l, H, DK).transpose(0, 2, 1, 3)
    v = (query @ W_v).reshape(b, l, H, DK).transpose(0, 2, 1, 3)
    s = np.einsum("bhqd,bhkd->bhqk", q, k) / np.sqrt(DK)
    s = np.where(mask, np.float32(-1e9), s)
    s = s - s.max(-1, keepdims=True)
    e = np.exp(s)
    A_ = e / e.sum(-1, keepdims=True)
    out = np.einsum("bhqk,bhkd->bhqd", A_, v)
    out = out.transpose(0, 2, 1, 3).reshape(b, l, d)
    return (out @ W_o).astype(np.float32), A_.astype(np.float32)


def kernel(query, mask, W_q, W_k, W_v, W_o):
    query = np.ascontiguousarray(np.asarray(query, dtype=np.float32))
    W_q = np.asarray(W_q, dtype=np.float32)
    W_k = np.asarray(W_k, dtype=np.float32)
    W_v = np.asarray(W_v, dtype=np.float32)
    W_o = np.asarray(W_o, dtype=np.float32)
    mask = np.asarray(mask)

    causal = np.array_equal(
        np.asarray(mask).reshape(L, L),
        np.triu(np.ones((L, L), dtype=bool), k=1),
    )
    if not causal:
        return _reference_np(query, mask, W_q, W_k, W_v, W_o)

    from concourse.bass_utils import run_bass_kernel_spmd

    nc = _get_prog()
    xh = query.reshape(TOK, D)
    mtri = (np.arange(128)[:, None] <= np.arange(128)[None, :]).astype(np.float32)
    mtril = mtri.T.copy()
    in_maps = []
    for c in range(NC):
        c0 = c * 128
        in_maps.append({
            "x": xh,
            "wq": np.ascontiguousarray(W_q[:, c0:c0 + 128]),
            "wk": np.ascontiguousarray(W_k[:, c0:c0 + 128]),
            "wv": np.ascontiguousarray(W_v[:, c0:c0 + 128]),
            "wo": np.ascontiguousarray(W_o[c0:c0 + 128, :]),
            "mtri": mtri, "mtril": mtril,
        })
    res = run_bass_kernel_spmd(nc, in_maps, list(range(NC))).results
    yfull = res[0]["y"].astype(np.float32)
    for c in range(1, NC):
        yfull = yfull + res[c]["y"]
    Afull = np.concatenate([res[c]["A"] for c in range(NC)], axis=1)
    return yfull.reshape(B, L, D), Afull


# revision 14
# speedup vs baseline: 2.1576x; 2.1576x over previous
"""MHA kernel for TRN2, 8-core SPMD (head-parallel).

Sharding: each core owns 2 of the 16 heads (column slices of W_q/W_k/W_v,
row slice of W_o). Each core computes its heads' Q/K/V for all tokens,
causal attention (lower-triangular blocks only; softmax without max
subtraction since scores ~ N(0,1)), its A shard, and a partial y
(contraction over its 128 head dims). Host sums y partials and concatenates
A shards.

Matmuls run in float32r (tf32-class, ~1.5e-4 rel err, 4x the fp32 rate).
"""
import sys

sys.path.insert(0, "/opt/trn_rl_repo")

import numpy as np

B, L, D, H = 2, 2048, 1024, 16
DK = D // H          # 64
NC = 8               # cores
HPC = H // NC        # 2 heads per core
TOK = B * L          # 4096
CHUNK = 512          # token chunk (phase 1) / q chunk (phase 2)
KT = 128             # k tile
NQC = L // CHUNK     # 4 q-chunks per batch
NKTB = L // KT       # 16 k-tiles per batch

_prog = None


def _build():
    import concourse.bacc as bacc
    import concourse.mybir as mybir
    import concourse.tile as tile
    from concourse.masks import make_identity

    f32 = mybir.dt.float32
    f32r = mybir.dt.float32r
    bf16 = mybir.dt.bfloat16
    ptdt = f32r          # dtype of exp(P) tiles / PV operands
    Exp = mybir.ActivationFunctionType.Exp
    Copy = mybir.ActivationFunctionType.Copy

    nc = bacc.Bacc(None, target_bir_lowering=False)
    x = nc.dram_tensor("x", [TOK, D], f32, kind="ExternalInput")
    wq = nc.dram_tensor("wq", [D, NC * HPC * DK // NC], f32, kind="ExternalInput")
    wk = nc.dram_tensor("wk", [D, 128], f32, kind="ExternalInput")
    wv = nc.dram_tensor("wv", [D, 128], f32, kind="ExternalInput")
    wo = nc.dram_tensor("wo", [128, D], f32, kind="ExternalInput")
    mtri = nc.dram_tensor("mtri", [128, 128], f32, kind="ExternalInput")
    mtril = nc.dram_tensor("mtril", [128, 128], f32, kind="ExternalInput")
    A = nc.dram_tensor("A", [B, HPC, L, L], f32, kind="ExternalOutput")
    y = nc.dram_tensor("y", [TOK, D], f32, kind="ExternalOutput")

    with tile.TileContext(nc) as tc:
        with tc.tile_pool(name="consts", bufs=1) as consts, \
             tc.tile_pool(name="persist", bufs=1) as persist:
            identf = consts.tile([128, 128], f32)
            make_identity(nc, identf[:, :])
            identr = consts.tile([128, 128], f32r)
            nc.vector.tensor_copy(identr[:, :], identf[:, :])
            identp = consts.tile([128, 128], ptdt)
            nc.vector.tensor_copy(identp[:, :], identf[:, :])
            mtri_sb = consts.tile([128, 128], f32)
            nc.sync.dma_start(out=mtri_sb[:, :], in_=mtri.ap())
            mtril_sb = consts.tile([128, 128], f32)
            nc.sync.dma_start(out=mtril_sb[:, :], in_=mtril.ap())
            ones_f = consts.tile([1, 128], f32)
            nc.vector.memset(ones_f[:, :], 1.0)
            ones_r = consts.tile([1, 128], f32r)
            nc.vector.tensor_copy(ones_r[:, :], ones_f[:, :])
            ones_col = consts.tile([128, 1], f32)
            nc.vector.memset(ones_col[:, :], 1.0)

            # weights -> f32r  (lhsT layout: [k-tile-row 128, k-tile, cols])
            wq_r = persist.tile([128, 8, 128], f32r)
            wk_r = persist.tile([128, 8, 128], f32r)
            wv_r = persist.tile([128, 8, 128], f32r)
            wo_r = persist.tile([128, D], f32r)
            qt_sb = persist.tile([128, B, L], f32r)   # Q^T  (2 heads stacked)
            kt_sb = persist.tile([128, B, L], f32r)   # K^T
            v_sb = persist.tile([128, B, NKTB, HPC, DK + 1], ptdt)  # [V|1]

            with tc.tile_pool(name="p1sb", bufs=2) as p1sb, \
                 tc.tile_pool(name="p1ps", bufs=1, space="PSUM") as p1ps:
                for wdram, wr in ((wq, wq_r), (wk, wk_r), (wv, wv_r)):
                    wst = p1sb.tile([128, 8, 128], f32, tag="wst")
                    nc.sync.dma_start(
                        out=wst[:, :, :],
                        in_=wdram.ap().rearrange("(kt p) m -> p kt m", p=128),
                    )
                    nc.vector.tensor_copy(wr[:, :, :], wst[:, :, :])
                wst = p1sb.tile([128, 8, 128], f32, tag="wst")
                nc.sync.dma_start(
                    out=wst[:, :, :],
                    in_=wo.ap().rearrange("p (kt m) -> p kt m", kt=8),
                )
                nc.vector.tensor_copy(
                    wo_r.rearrange("p (kt m) -> p kt m", kt=8), wst[:, :, :]
                )

                # ones column of [V|1]
                for b in range(B):
                    for kt in range(NKTB):
                        for hh in range(HPC):
                            nc.vector.tensor_copy(
                                v_sb[:, b, kt, hh, DK:DK + 1], ones_col[:, :]
                            )

                # ---- phase 1: x -> x^T -> Q^T/K^T/V ----
                for ci in range(TOK // CHUNK):
                    b = ci // NQC
                    cb = ci % NQC  # chunk within batch
                    xst = p1sb.tile([128, 4, D], f32, tag="xst")
                    nc.sync.dma_start(
                        out=xst[:, :, :],
                        in_=x.ap()[ci * CHUNK:(ci + 1) * CHUNK, :].rearrange(
                            "(n p) d -> p n d", p=128
                        ),
                    )
                    xtc = p1sb.tile([128, 8, CHUNK], f32r, tag="xtc")
                    for kt in range(8):
                        for t in range(4):
                            tp = p1ps.tile([128, 128], f32, tag="tp", bufs=4)
                            nc.tensor.transpose(
                                tp[:, :], xst[:, t, kt * 128:(kt + 1) * 128],
                                identf[:, :],
                            )
                            eng = nc.scalar if (kt + t) % 2 else nc.vector
                            if eng is nc.scalar:
                                nc.scalar.copy(
                                    out=xtc[:, kt, t * 128:(t + 1) * 128],
                                    in_=tp[:, :],
                                )
                            else:
                                nc.vector.tensor_copy(
                                    xtc[:, kt, t * 128:(t + 1) * 128], tp[:, :]
                                )
                    for wr, dst in ((wq_r, qt_sb), (wk_r, kt_sb)):
                        pp = p1ps.tile([128, CHUNK], f32, tag="proj", bufs=3)
                        for kt in range(8):
                            nc.tensor.matmul(
                                pp[:, :], wr[:, kt, :], xtc[:, kt, :],
                                start=(kt == 0), stop=(kt == 7),
                            )
                        nc.scalar.copy(
                            out=dst[:, b, cb * CHUNK:(cb + 1) * CHUNK], in_=pp[:, :]
                        )
                    # V: project then transpose to [tokens, dims]
                    pp = p1ps.tile([128, CHUNK], f32, tag="proj", bufs=3)
                    for kt in range(8):
                        nc.tensor.matmul(
                            pp[:, :], wv_r[:, kt, :], xtc[:, kt, :],
                            start=(kt == 0), stop=(kt == 7),
                        )
                    vtmp = p1sb.tile([128, CHUNK], f32, tag="vtmp")
                    nc.vector.tensor_copy(vtmp[:, :], pp[:, :])
                    for t in range(4):
                        vp = p1ps.tile([128, 128], f32, tag="vtp")
                        nc.tensor.transpose(
                            vp[:, :], vtmp[:, t * 128:(t + 1) * 128], identf[:, :]
                        )
                        ktile = cb * 4 + t
                        for hh in range(HPC):
                            nc.vector.tensor_copy(
                                v_sb[:, b, ktile, hh, 0:DK],
                                vp[:, hh * DK:(hh + 1) * DK],
                            )

            # ---- phase 2: attention + Wo ----
            with tc.tile_pool(name="p2sb", bufs=1) as p2sb, \
                 tc.tile_pool(name="ptp", bufs=16) as ptp, \
                 tc.tile_pool(name="aasm", bufs=6) as aasm, \
                 tc.tile_pool(name="p2x", bufs=2) as p2x, \
                 tc.tile_pool(name="rqp", bufs=4) as rqp, \
                 tc.tile_pool(name="ysb", bufs=3) as ysbp, \
                 tc.tile_pool(name="p2ps", bufs=1, space="PSUM") as p2ps:
                for b in range(B):
                    for qc in range(NQC):
                        q0 = qc * CHUNK
                        nkt = 4 * (qc + 1)
                        otn = p2x.tile([128, CHUNK], f32r, tag="otn")
                        # stage A: S + exp, heads interleaved so consecutive S
                        # matmuls hit alternating PE row groups (0-63 / 64-127)
                        # and LDWEIGHTS can pull ahead of in-flight matmuls.
                        pts_h = [[], []]
                        for kt in range(nkt):
                            k0 = kt * KT
                            off = max(0, k0 - q0)
                            for h in range(HPC):
                                r0, r1 = h * DK, (h + 1) * DK
                                s_ps = p2ps.tile([128, CHUNK], f32, tag="sps", bufs=2)
                                nc.tensor.matmul(
                                    s_ps[:, off:CHUNK],
                                    kt_sb[r0:r1, b, k0:k0 + KT],
                                    qt_sb[r0:r1, b, q0 + off:q0 + CHUNK],
                                    start=True, stop=True,
                                )
                                pt = ptp.tile([128, CHUNK], ptdt, tag="pt", bufs=32)
                                nc.scalar.activation(
                                    out=pt[:, off:CHUNK], in_=s_ps[:, off:CHUNK],
                                    func=Exp, scale=0.125,
                                )
                                if k0 >= q0:  # diagonal tile: causal tri-mask
                                    nc.vector.tensor_mul(
                                        pt[:, off:off + KT], pt[:, off:off + KT],
                                        mtri_sb[:, :],
                                    )
                                pts_h[h].append(pt)
                        # stage B: PV, heads interleaved (alternating PSUM banks)
                        o_ps_h = []
                        for h in range(HPC):
                            o_ps = p2ps.tile([DK + 1, CHUNK], f32, tag="oy",
                                             bufs=2, name=f"ops_{b}_{qc}_{h}")
                            o_ps_h.append(o_ps)
                        for kt in range(nkt):
                            off = max(0, kt * KT - q0)
                            for h in range(HPC):
                                nc.tensor.matmul(
                                    o_ps_h[h][:, off:CHUNK],
                                    v_sb[:, b, kt, h, :],
                                    pts_h[h][kt][:, off:CHUNK],
                                    start=(kt == 0), stop=(kt == nkt - 1),
                                )
                        rq_h = [[], []]
                        rs_h = []
                        for h in range(HPC):
                            r0, r1 = h * DK, (h + 1) * DK
                            o_ps = o_ps_h[h]
                            zrow = p2x.tile([1, CHUNK], f32, tag="zrow")
                            nc.vector.tensor_copy(zrow[:, :], o_ps[DK:DK + 1, :])
                            r_sb = p2x.tile([1, CHUNK], f32, tag="rsb")
                            nc.vector.reciprocal_approx_fast(
                                out=r_sb[:, :], in_=zrow[:, :]
                            )
                            rs_h.append(r_sb)
                            r_r = p2x.tile([1, CHUNK], f32r, tag="rr")
                            nc.vector.tensor_copy(r_r[:, :], r_sb[:, :])
                            bc_ps = p2ps.tile([DK, CHUNK], f32, tag="bc")
                            nc.tensor.matmul(
                                bc_ps[:, :], ones_r[:, 0:DK], r_r[:, :],
                                start=True, stop=True,
                            )
                            bc_sb = p2x.tile([DK, CHUNK], f32, tag="bcsb")
                            nc.vector.tensor_copy(bc_sb[:, :], bc_ps[:, :])
                            nc.vector.tensor_mul(
                                otn[r0:r1, :], o_ps[0:DK, :], bc_sb[:, :]
                            )
                            for t in range(4):
                                zc_ps = p2ps.tile([128, CHUNK], f32, tag="sps",
                                                  bufs=2)
                                nc.tensor.matmul(
                                    zc_ps[:, 0:1], r_sb[:, t * 128:(t + 1) * 128],
                                    ones_f[:, 0:1], start=True, stop=True,
                                )
                                rq = rqp.tile([128, 1], f32, tag="rq", bufs=8)
                                nc.vector.tensor_copy(rq[:, :], zc_ps[:, 0:1])
                                rq_h[h].append(rq)
                        # stage C: A shard via S-natural recompute; normalize
                        # folded into exp as a per-partition ln(1/Z) bias.
                        for t in range(4):
                            a_asm_h = []
                            for h in range(HPC):
                                a_asm = aasm.tile([128, L], f32, tag="aasm",
                                                  name=f"aasm_{b}_{qc}_{t}_{h}")
                                a_asm_h.append(a_asm)
                            ncols = q0 + t * 128 + 128
                            qa, qb2 = q0 + t * 128, q0 + t * 128 + 128
                            for c0 in range(0, ncols, CHUNK):
                                cw = min(CHUNK, ncols - c0)
                                for h in range(HPC):
                                    r0, r1 = h * DK, (h + 1) * DK
                                    sn_ps = p2ps.tile([128, CHUNK], f32,
                                                      tag="snat", bufs=3,
                                                      name=f"sn_{h}")
                                    nc.tensor.matmul(
                                        sn_ps[:, 0:cw],
                                        qt_sb[r0:r1, b, qa:qb2],
                                        kt_sb[r0:r1, b, c0:c0 + cw],
                                        start=True, stop=True,
                                    )
                                    nc.scalar.activation(
                                        out=a_asm_h[h][:, c0:c0 + cw],
                                        in_=sn_ps[:, 0:cw],
                                        func=Exp, scale=0.125,
                                    )
                                    nc.vector.tensor_scalar_mul(
                                        a_asm_h[h][:, c0:c0 + cw],
                                        a_asm_h[h][:, c0:c0 + cw],
                                        rq_h[h][t][:, :],
                                    )
                            for h in range(HPC):
                                # causal mask on the diagonal 128-block
                                nc.vector.tensor_mul(
                                    a_asm_h[h][:, qa:qb2],
                                    a_asm_h[h][:, qa:qb2], mtril_sb[:, :],
                                )
                                nc.sync.dma_start(
                                    out=A.ap()[b, h, qa:qb2, 0:ncols],
                                    in_=a_asm_h[h][:, 0:ncols],
                                )
                        # Wo: y tile = OtN.T @ wo_rows (partial over this core's dims)
                        for t in range(4):
                            for n in range(2):
                                y_ps = p2ps.tile([128, CHUNK], f32, tag="oy", bufs=2)
                                nc.tensor.matmul(
                                    y_ps[:, :],
                                    otn[:, t * 128:(t + 1) * 128],
                                    wo_r[:, n * CHUNK:(n + 1) * CHUNK],
                                    start=True, stop=True,
                                )
                                y_sb = ysbp.tile([128, CHUNK], f32, tag="ysb")
                                nc.vector.tensor_copy(y_sb[:, :], y_ps[:, :])
                                g0 = b * L + q0 + t * 128
                                nc.sync.dma_start(
                                    out=y.ap()[g0:g0 + 128,
                                               n * CHUNK:(n + 1) * CHUNK],
                                    in_=y_sb[:, :],
                                )
    nc.compile()
    return nc


def _get_prog():
    global _prog
    if _prog is None:
        _prog = _build()
    return _prog


def _reference_np(query, mask, W_q, W_k, W_v, W_o):
    b, l, d = query.shape
    q = (query @ W_q).reshape(b, l, H, DK).transpose(0, 2, 1, 3)
    k = (query @ W_k).reshape(b, l, H, DK).transpose(0, 2, 1, 3)
    v = (query @ W_v).reshape(b, l, H, DK).transpose(0, 2, 1, 3)
    s = np.einsum("bhqd,bhkd->bhqk", q, k) / np.sqrt(DK)
    s = np.where(mask, np.float32(-1e9), s)
    s = s - s.max(-1, keepdims=True)
    e = np.exp(s)
    A_ = e / e.sum(-1, keepdims=True)
    out = np.einsum("bhqk,bhkd->bhqd", A_, v)
    out = out.transpose(0, 2, 1, 3).reshape(b, l, d)
    return (out @ W_o).astype(np.float32), A_.astype(np.float32)


def kernel(query, mask, W_q, W_k, W_v, W_o):
    query = np.ascontiguousarray(np.asarray(query, dtype=np.float32))
    W_q = np.asarray(W_q, dtype=np.float32)
    W_k = np.asarray(W_k, dtype=np.float32)
    W_v = np.asarray(W_v, dtype=np.float32)
    W_o = np.asarray(W_o, dtype=np.float32)
    mask = np.asarray(mask)

    causal = np.array_equal(
        np.asarray(mask).reshape(L, L),
        np.triu(np.ones((L, L), dtype=bool), k=1),
    )
    if not causal:
        return _reference_np(query, mask, W_q, W_k, W_v, W_o)

    from concourse.bass_utils import run_bass_kernel_spmd

    nc = _get_prog()
    xh = query.reshape(TOK, D)
    mtri = (np.arange(128)[:, None] <= np.arange(128)[None, :]).astype(np.float32)
    mtril = mtri.T.copy()
    in_maps = []
    for c in range(NC):
        c0 = c * 128
        in_maps.append({
            "x": xh,
            "wq": np.ascontiguousarray(W_q[:, c0:c0 + 128]),
            "wk": np.ascontiguousarray(W_k[:, c0:c0 + 128]),
            "wv": np.ascontiguousarray(W_v[:, c0:c0 + 128]),
            "wo": np.ascontiguousarray(W_o[c0:c0 + 128, :]),
            "mtri": mtri, "mtril": mtril,
        })
    res = run_bass_kernel_spmd(nc, in_maps, list(range(NC))).results
    yfull = res[0]["y"].astype(np.float32)
    for c in range(1, NC):
        yfull = yfull + res[c]["y"]
    Afull = np.concatenate([res[c]["A"] for c in range(NC)], axis=1)
    return yfull.reshape(B, L, D), Afull


# revision 17
# speedup vs baseline: 2.1794x; 1.0101x over previous
"""MHA kernel for TRN2, 8-core SPMD (head-parallel).

Sharding: each core owns 2 of the 16 heads (column slices of W_q/W_k/W_v,
row slice of W_o). Each core computes its heads' Q/K/V for all tokens,
causal attention (lower-triangular blocks only; softmax without max
subtraction since scores ~ N(0,1)), its A shard, and a partial y
(contraction over its 128 head dims). Host sums y partials and concatenates
A shards.

Matmuls run in float32r (tf32-class, ~1.5e-4 rel err, 4x the fp32 rate).
"""
import sys

sys.path.insert(0, "/opt/trn_rl_repo")

import numpy as np

B, L, D, H = 2, 2048, 1024, 16
DK = D // H          # 64
NC = 8               # cores
HPC = H // NC        # 2 heads per core
TOK = B * L          # 4096
CHUNK = 512          # token chunk (phase 1) / q chunk (phase 2)
KT = 128             # k tile
NQC = L // CHUNK     # 4 q-chunks per batch
NKTB = L // KT       # 16 k-tiles per batch

_prog = None


def _build():
    import concourse.bacc as bacc
    import concourse.mybir as mybir
    import concourse.tile as tile
    from concourse.masks import make_identity

    f32 = mybir.dt.float32
    f32r = mybir.dt.float32r
    bf16 = mybir.dt.bfloat16
    ptdt = f32r          # dtype of exp(P) tiles / PV operands
    Exp = mybir.ActivationFunctionType.Exp
    Copy = mybir.ActivationFunctionType.Copy

    nc = bacc.Bacc(None, target_bir_lowering=False)
    x = nc.dram_tensor("x", [TOK, D], f32, kind="ExternalInput")
    wq = nc.dram_tensor("wq", [D, NC * HPC * DK // NC], f32, kind="ExternalInput")
    wk = nc.dram_tensor("wk", [D, 128], f32, kind="ExternalInput")
    wv = nc.dram_tensor("wv", [D, 128], f32, kind="ExternalInput")
    wo = nc.dram_tensor("wo", [128, D], f32, kind="ExternalInput")
    mtri = nc.dram_tensor("mtri", [128, 128], f32, kind="ExternalInput")
    mtril = nc.dram_tensor("mtril", [128, 128], f32, kind="ExternalInput")
    A = nc.dram_tensor("A", [B, HPC, L, L], f32, kind="ExternalOutput")
    y = nc.dram_tensor("y", [TOK, D], f32, kind="ExternalOutput")

    with tile.TileContext(nc) as tc:
        with tc.tile_pool(name="consts", bufs=1) as consts, \
             tc.tile_pool(name="persist", bufs=1) as persist:
            identf = consts.tile([128, 128], f32)
            make_identity(nc, identf[:, :])
            identr = consts.tile([128, 128], f32r)
            nc.vector.tensor_copy(identr[:, :], identf[:, :])
            identp = consts.tile([128, 128], ptdt)
            nc.vector.tensor_copy(identp[:, :], identf[:, :])
            mtri_sb = consts.tile([128, 128], f32)
            nc.sync.dma_start(out=mtri_sb[:, :], in_=mtri.ap())
            mtril_sb = consts.tile([128, 128], f32)
            nc.sync.dma_start(out=mtril_sb[:, :], in_=mtril.ap())
            ones_f = consts.tile([1, 128], f32)
            nc.vector.memset(ones_f[:, :], 1.0)
            ones_r = consts.tile([1, 128], f32r)
            nc.vector.tensor_copy(ones_r[:, :], ones_f[:, :])
            ones_col = consts.tile([128, 1], f32)
            nc.vector.memset(ones_col[:, :], 1.0)

            # weights -> f32r  (lhsT layout: [k-tile-row 128, k-tile, cols])
            wq_r = persist.tile([128, 8, 128], f32r)
            wk_r = persist.tile([128, 8, 128], f32r)
            wv_r = persist.tile([128, 8, 128], f32r)
            wo_r = persist.tile([128, D], f32r)
            qt_sb = persist.tile([128, B, L], f32r)   # Q^T  (2 heads stacked)
            kt_sb = persist.tile([128, B, L], f32r)   # K^T
            v_sb = persist.tile([128, B, NKTB, HPC, DK + 1], ptdt)  # [V|1]

            with tc.tile_pool(name="p1sb", bufs=2) as p1sb, \
                 tc.tile_pool(name="p1ps", bufs=1, space="PSUM") as p1ps:
                for wdram, wr in ((wq, wq_r), (wk, wk_r), (wv, wv_r)):
                    wst = p1sb.tile([128, 8, 128], f32, tag="wst")
                    nc.sync.dma_start(
                        out=wst[:, :, :],
                        in_=wdram.ap().rearrange("(kt p) m -> p kt m", p=128),
                    )
                    nc.vector.tensor_copy(wr[:, :, :], wst[:, :, :])
                wst = p1sb.tile([128, 8, 128], f32, tag="wst")
                nc.sync.dma_start(
                    out=wst[:, :, :],
                    in_=wo.ap().rearrange("p (kt m) -> p kt m", kt=8),
                )
                nc.vector.tensor_copy(
                    wo_r.rearrange("p (kt m) -> p kt m", kt=8), wst[:, :, :]
                )

                # ones column of [V|1]
                for b in range(B):
                    for kt in range(NKTB):
                        for hh in range(HPC):
                            nc.vector.tensor_copy(
                                v_sb[:, b, kt, hh, DK:DK + 1], ones_col[:, :]
                            )

                # ---- phase 1: x -> x^T -> Q^T/K^T/V ----
                for ci in range(TOK // CHUNK):
                    b = ci // NQC
                    cb = ci % NQC  # chunk within batch
                    xst = p1sb.tile([128, 4, D], f32, tag="xst")
                    nc.sync.dma_start(
                        out=xst[:, :, :],
                        in_=x.ap()[ci * CHUNK:(ci + 1) * CHUNK, :].rearrange(
                            "(n p) d -> p n d", p=128
                        ),
                    )
                    xtc = p1sb.tile([128, 8, CHUNK], f32r, tag="xtc")
                    for kt in range(8):
                        for t in range(4):
                            tp = p1ps.tile([128, 128], f32, tag="tp", bufs=4)
                            nc.tensor.transpose(
                                tp[:, :], xst[:, t, kt * 128:(kt + 1) * 128],
                                identf[:, :],
                            )
                            eng = nc.scalar if (kt + t) % 2 else nc.vector
                            if eng is nc.scalar:
                                nc.scalar.copy(
                                    out=xtc[:, kt, t * 128:(t + 1) * 128],
                                    in_=tp[:, :],
                                )
                            else:
                                nc.vector.tensor_copy(
                                    xtc[:, kt, t * 128:(t + 1) * 128], tp[:, :]
                                )
                    for wr, dst in ((wq_r, qt_sb), (wk_r, kt_sb)):
                        pp = p1ps.tile([128, CHUNK], f32, tag="proj", bufs=3)
                        for kt in range(8):
                            nc.tensor.matmul(
                                pp[:, :], wr[:, kt, :], xtc[:, kt, :],
                                start=(kt == 0), stop=(kt == 7),
                            )
                        nc.scalar.copy(
                            out=dst[:, b, cb * CHUNK:(cb + 1) * CHUNK], in_=pp[:, :]
                        )
                    # V: project then transpose to [tokens, dims]
                    pp = p1ps.tile([128, CHUNK], f32, tag="proj", bufs=3)
                    for kt in range(8):
                        nc.tensor.matmul(
                            pp[:, :], wv_r[:, kt, :], xtc[:, kt, :],
                            start=(kt == 0), stop=(kt == 7),
                        )
                    vtmp = p1sb.tile([128, CHUNK], f32, tag="vtmp")
                    nc.vector.tensor_copy(vtmp[:, :], pp[:, :])
                    for t in range(4):
                        vp = p1ps.tile([128, 128], f32, tag="vtp")
                        nc.tensor.transpose(
                            vp[:, :], vtmp[:, t * 128:(t + 1) * 128], identf[:, :]
                        )
                        ktile = cb * 4 + t
                        for hh in range(HPC):
                            nc.vector.tensor_copy(
                                v_sb[:, b, ktile, hh, 0:DK],
                                vp[:, hh * DK:(hh + 1) * DK],
                            )

            # ---- phase 2: attention + Wo ----
            with tc.tile_pool(name="p2sb", bufs=1) as p2sb, \
                 tc.tile_pool(name="ptp", bufs=16) as ptp, \
                 tc.tile_pool(name="aasm", bufs=6) as aasm, \
                 tc.tile_pool(name="p2x", bufs=2) as p2x, \
                 tc.tile_pool(name="rqp", bufs=4) as rqp, \
                 tc.tile_pool(name="ysb", bufs=3) as ysbp, \
                 tc.tile_pool(name="p2ps", bufs=1, space="PSUM") as p2ps:
                for b in range(B):
                    for qc in range(NQC):
                        q0 = qc * CHUNK
                        nkt = 4 * (qc + 1)
                        otn = p2x.tile([128, CHUNK], f32r, tag="otn")
                        # stages A+B interleaved: S + exp per k-tile, with the
                        # PV matmul for k-tile (kt - LAG) woven into the same
                        # PE stream — PV inputs are exp'd by then, so the PE
                        # never stalls on ACT; heads alternate row groups so
                        # LDWEIGHTS pulls ahead.
                        LAG = 2
                        pts_h = [[], []]
                        o_ps_h = []
                        for h in range(HPC):
                            o_ps = p2ps.tile([DK + 1, CHUNK], f32, tag="oy",
                                             bufs=2, name=f"ops_{b}_{qc}_{h}")
                            o_ps_h.append(o_ps)

                        def emit_pv(kt):
                            off = max(0, kt * KT - q0)
                            for h in range(HPC):
                                nc.tensor.matmul(
                                    o_ps_h[h][:, off:CHUNK],
                                    v_sb[:, b, kt, h, :],
                                    pts_h[h][kt][:, off:CHUNK],
                                    start=(kt == 0), stop=(kt == nkt - 1),
                                )

                        for kt in range(nkt):
                            k0 = kt * KT
                            off = max(0, k0 - q0)
                            for h in range(HPC):
                                r0, r1 = h * DK, (h + 1) * DK
                                s_ps = p2ps.tile([128, CHUNK], f32, tag="sps", bufs=3)
                                nc.tensor.matmul(
                                    s_ps[:, off:CHUNK],
                                    kt_sb[r0:r1, b, k0:k0 + KT],
                                    qt_sb[r0:r1, b, q0 + off:q0 + CHUNK],
                                    start=True, stop=True,
                                )
                                pt = ptp.tile([128, CHUNK], ptdt, tag="pt", bufs=32)
                                nc.scalar.activation(
                                    out=pt[:, off:CHUNK], in_=s_ps[:, off:CHUNK],
                                    func=Exp, scale=0.125,
                                )
                                if k0 >= q0:  # diagonal tile: causal tri-mask
                                    nc.vector.tensor_mul(
                                        pt[:, off:off + KT], pt[:, off:off + KT],
                                        mtri_sb[:, :],
                                    )
                                pts_h[h].append(pt)
                            if kt >= LAG:
                                emit_pv(kt - LAG)
                        for kt in range(max(0, nkt - LAG), nkt):
                            emit_pv(kt)
                        rq_h = [[], []]
                        rs_h = []
                        for h in range(HPC):
                            r0, r1 = h * DK, (h + 1) * DK
                            o_ps = o_ps_h[h]
                            zrow = p2x.tile([1, CHUNK], f32, tag="zrow")
                            nc.vector.tensor_copy(zrow[:, :], o_ps[DK:DK + 1, :])
                            r_sb = p2x.tile([1, CHUNK], f32, tag="rsb")
                            nc.vector.reciprocal_approx_fast(
                                out=r_sb[:, :], in_=zrow[:, :]
                            )
                            rs_h.append(r_sb)
                            r_r = p2x.tile([1, CHUNK], f32r, tag="rr")
                            nc.vector.tensor_copy(r_r[:, :], r_sb[:, :])
                            bc_ps = p2ps.tile([DK, CHUNK], f32, tag="bc")
                            nc.tensor.matmul(
                                bc_ps[:, :], ones_r[:, 0:DK], r_r[:, :],
                                start=True, stop=True,
                            )
                            bc_sb = p2x.tile([DK, CHUNK], f32, tag="bcsb")
                            nc.vector.tensor_copy(bc_sb[:, :], bc_ps[:, :])
                            nc.vector.tensor_mul(
                                otn[r0:r1, :], o_ps[0:DK, :], bc_sb[:, :]
                            )
                            for t in range(4):
                                zc_ps = p2ps.tile([128, CHUNK], f32, tag="sps",
                                                  bufs=3)
                                nc.tensor.matmul(
                                    zc_ps[:, 0:1], r_sb[:, t * 128:(t + 1) * 128],
                                    ones_f[:, 0:1], start=True, stop=True,
                                )
                                rq = rqp.tile([128, 1], f32, tag="rq", bufs=8)
                                nc.vector.tensor_copy(rq[:, :], zc_ps[:, 0:1])
                                rq_h[h].append(rq)
                        # stage C: A shard via S-natural recompute; normalize
                        # folded into exp as a per-partition ln(1/Z) bias.
                        for t in range(4):
                            a_asm_h = []
                            for h in range(HPC):
                                a_asm = aasm.tile([128, L], f32, tag="aasm",
                                                  name=f"aasm_{b}_{qc}_{t}_{h}")
                                a_asm_h.append(a_asm)
                            ncols = q0 + t * 128 + 128
                            qa, qb2 = q0 + t * 128, q0 + t * 128 + 128
                            for c0 in range(0, ncols, CHUNK):
                                cw = min(CHUNK, ncols - c0)
                                for h in range(HPC):
                                    r0, r1 = h * DK, (h + 1) * DK
                                    sn_ps = p2ps.tile([128, CHUNK], f32,
                                                      tag="snat", bufs=2,
                                                      name=f"sn_{h}")
                                    nc.tensor.matmul(
                                        sn_ps[:, 0:cw],
                                        qt_sb[r0:r1, b, qa:qb2],
                                        kt_sb[r0:r1, b, c0:c0 + cw],
                                        start=True, stop=True,
                                    )
                                    nc.scalar.activation(
                                        out=a_asm_h[h][:, c0:c0 + cw],
                                        in_=sn_ps[:, 0:cw],
                                        func=Exp, scale=0.125,
                                    )
                                    nc.vector.tensor_scalar_mul(
                                        a_asm_h[h][:, c0:c0 + cw],
                                        a_asm_h[h][:, c0:c0 + cw],
                                        rq_h[h][t][:, :],
                                    )
                            for h in range(HPC):
                                # causal mask on the diagonal 128-block
                                nc.vector.tensor_mul(
                                    a_asm_h[h][:, qa:qb2],
                                    a_asm_h[h][:, qa:qb2], mtril_sb[:, :],
                                )
                                nc.sync.dma_start(
                                    out=A.ap()[b, h, qa:qb2, 0:ncols],
                                    in_=a_asm_h[h][:, 0:ncols],
                                )
                        # Wo: y tile = OtN.T @ wo_rows (partial over this core's dims)
                        for t in range(4):
                            for n in range(2):
                                y_ps = p2ps.tile([128, CHUNK], f32, tag="oy", bufs=2)
                                nc.tensor.matmul(
                                    y_ps[:, :],
                                    otn[:, t * 128:(t + 1) * 128],
                                    wo_r[:, n * CHUNK:(n + 1) * CHUNK],
                                    start=True, stop=True,
                                )
                                y_sb = ysbp.tile([128, CHUNK], f32, tag="ysb")
                                nc.vector.tensor_copy(y_sb[:, :], y_ps[:, :])
                                g0 = b * L + q0 + t * 128
                                nc.sync.dma_start(
                                    out=y.ap()[g0:g0 + 128,
                                               n * CHUNK:(n + 1) * CHUNK],
                                    in_=y_sb[:, :],
                                )
    nc.compile()
    return nc


def _get_prog():
    global _prog
    if _prog is None:
        _prog = _build()
    return _prog


def _reference_np(query, mask, W_q, W_k, W_v, W_o):
    b, l, d = query.shape
    q = (query @ W_q).reshape(b, l, H, DK).transpose(0, 2, 1, 3)
    k = (query @ W_k).reshape(b, l, H, DK).transpose(0, 2, 1, 3)
    v = (query @ W_v).reshape(b, l, H, DK).transpose(0, 2, 1, 3)
    s = np.einsum("bhqd,bhkd->bhqk", q, k) / np.sqrt(DK)
    s = np.where(mask, np.float32(-1e9), s)
    s = s - s.max(-1, keepdims=True)
    e = np.exp(s)
    A_ = e / e.sum(-1, keepdims=True)
    out = np.einsum("bhqk,bhkd->bhqd", A_, v)
    out = out.transpose(0, 2, 1, 3).reshape(b, l, d)
    return (out @ W_o).astype(np.float32), A_.astype(np.float32)


def kernel(query, mask, W_q, W_k, W_v, W_o):
    query = np.ascontiguousarray(np.asarray(query, dtype=np.float32))
    W_q = np.asarray(W_q, dtype=np.float32)
    W_k = np.asarray(W_k, dtype=np.float32)
    W_v = np.asarray(W_v, dtype=np.float32)
    W_o = np.asarray(W_o, dtype=np.float32)
    mask = np.asarray(mask)

    causal = np.array_equal(
        np.asarray(mask).reshape(L, L),
        np.triu(np.ones((L, L), dtype=bool), k=1),
    )
    if not causal:
        return _reference_np(query, mask, W_q, W_k, W_v, W_o)

    from concourse.bass_utils import run_bass_kernel_spmd

    nc = _get_prog()
    xh = query.reshape(TOK, D)
    mtri = (np.arange(128)[:, None] <= np.arange(128)[None, :]).astype(np.float32)
    mtril = mtri.T.copy()
    in_maps = []
    for c in range(NC):
        c0 = c * 128
        in_maps.append({
            "x": xh,
            "wq": np.ascontiguousarray(W_q[:, c0:c0 + 128]),
            "wk": np.ascontiguousarray(W_k[:, c0:c0 + 128]),
            "wv": np.ascontiguousarray(W_v[:, c0:c0 + 128]),
            "wo": np.ascontiguousarray(W_o[c0:c0 + 128, :]),
            "mtri": mtri, "mtril": mtril,
        })
    res = run_bass_kernel_spmd(nc, in_maps, list(range(NC))).results
    yfull = res[0]["y"].astype(np.float32)
    for c in range(1, NC):
        yfull = yfull + res[c]["y"]
    Afull = np.concatenate([res[c]["A"] for c in range(NC)], axis=1)
    return yfull.reshape(B, L, D), Afull
